# revision 16
# baseline (speedup 1.0000x reference)
"""Causal multi-head attention (B=4, S=2048, D=1024, H=16) on 8 NeuronCores.

Sharding: core c handles batch b=c//2 and head-group g=c%2 (8 heads, 512
features). Host pre-transposes x and the weight slices (all bf16) so every
device matmul contracts along the partition dim; the row-parallel
out-projection partials are summed pairwise on the host (+ bias).

Per-core pipeline (one Bass/Tile program, SPMD over 8 cores), fused at
q-block (512-token) granularity so TensorE never drains while the ACT
engine chews softmax exps:

  for qb in 0..3:
    1. q-projection for qb's tokens (bf16 matmuls, 8 dk chunks per m-tile).
    2. causal attention for qb against k-tiles 0..4qb+3, two heads per
       pair-stream packed at PE row offsets 0/64 so their K=64 score
       matmuls run concurrently in the array; one combined 2-head scores
       PSUM tile + a single exp per group keeps the pair adjacent in the
       PE queue. k/v-projections for qb's own tokens are emitted between
       pair-0's full groups as PE filler (only the diagonal needs them).
    3. denominators from an appended ones-column in the 65-wide V
       stationary; batched approx reciprocal, hi/lo bf16 split broadcast
       with K=2 PE outer products; normalize into bf16 ctxT.
    4. out-projection of qb's 4 token tiles; PSUM evicted by the ACT
       engine; f32 DMA out.
"""

import sys
import types

import numpy as np
import ml_dtypes

import concourse.bass as bass
import concourse.mybir as mybir
from concourse import tile
from concourse.bass_utils import run_bass_kernel_spmd
from concourse.masks import make_upper_triangular

# ----------------------------------------------------------------------------
# Compat patches for this container (self-contained on purpose).
# ----------------------------------------------------------------------------


def _patch_tail_drain():
    """This walrus build accepts only ONE sync-wait per sync-engine
    instruction; TileContext's tail drain may carry several. Split extras
    onto dedicated 1-wait nops."""
    from concourse.vector_clock import ScopedClock

    def _drain_and_barrier(self, tick_clock, wait_clock):
        nc = self.nc
        drain_inst = nc.sync.drain()
        wait_clock.add_sem_waits(
            drain_inst.ins, ScopedClock({None: tick_clock.global_clock})
        )
        si = drain_inst.ins.sync_info
        if si is not None and len(si.on_wait) > 1:
            waits = list(si.on_wait)
            drain_inst.ins.sync_info = mybir.SyncInfo(
                on_wait=waits[:1], on_update=list(si.on_update)
            )
            for w in waits[1:]:
                n = nc.sync.nop()
                n.ins.sync_info = mybir.SyncInfo(on_wait=[w], on_update=[])

        nc.all_engine_barrier()
        assert self.sems is not None
        popped = nc._tile_sem_poison_stack.pop()
        assert popped is self._sem_poison
        nc.clear_and_free_semaphores(list(self.sems.allocated().values()))
        nc.all_engine_barrier()

    tile.TileContext._drain_and_barrier = _drain_and_barrier


def _patch_profiling():
    """Provide the NTFF profile hook (image's antenv lacks axon_hooks) and
    disable cloud artifact uploads. Only matters when tracing is requested."""
    import concourse.bass_utils as bass_utils

    bass_utils.upload_artifacts = lambda tmpdir: tmpdir
    try:
        from antenv.axon_hooks import get_axon_ntff_profile_hook  # noqa: F401
        return
    except ImportError:
        pass
    try:
        from trn_agent_boot.trn_boot import _ntff_profile_via_ctypes

        hook = _ntff_profile_via_ctypes("/opt/axon/libaxon_pjrt.so")
    except Exception:
        hook = None
    mod = types.ModuleType("antenv.axon_hooks")
    mod._hook = hook
    mod.get_axon_ntff_profile_hook = lambda: mod._hook
    mod.set_axon_ntff_profile_hook = lambda h: setattr(mod, "_hook", h)
    sys.modules["antenv.axon_hooks"] = mod
    import antenv

    antenv.axon_hooks = mod


_patch_tail_drain()
_patch_profiling()


def _legalize_waits(nc):
    """This walrus build allows 1 sync-wait per instruction (2 on
    EventSemaphore). Split excess waits onto EventSemaphore carriers
    inserted just before the over-capacity instruction (same engine
    queue, so ordering semantics are preserved)."""
    n_fix = 0
    for f in nc.m.functions:
        for b in f.blocks:
            out = []
            changed = False
            for inst in b.instructions:
                si = inst.sync_info
                cap = 1
                if si is not None and len(si.on_wait) > cap:
                    waits = list(si.on_wait)
                    extra, keep = waits[:-cap], waits[-cap:]
                    for i in range(0, len(extra), 1):
                        n_fix += 1
                        out.append(
                            mybir.InstNoOp(
                                name=f"I-waitfix-{n_fix}",
                                engine=inst.engine,
                                ins=[],
                                outs=[],
                                sync_info=mybir.SyncInfo(
                                    on_wait=extra[i:i + 1], on_update=[]
                                ),
                            )
                        )
                    inst.sync_info = mybir.SyncInfo(
                        on_wait=keep, on_update=list(si.on_update)
                    )
                    changed = True
                out.append(inst)
            if changed:
                b.instructions = out

# ----------------------------------------------------------------------------
# Problem constants (hardcoded; kernel.py must be self-contained).
# ----------------------------------------------------------------------------
B, S, D, H = 4, 2048, 1024, 16
HD = D // H          # 64 head dim
NCORES = 8
GPC = 2              # head-groups per batch (cores per batch)
FPC = D // GPC       # 512 features per core
HPC = H // GPC       # 8 heads per core
P = 128
DC = D // P          # 8 contraction chunks
NT = S // P          # 16 token tiles
QB = 512             # q-block
NQB = S // QB        # 4
VW = 66              # vtm row width: 64 v dims + ones col + pad

F32 = mybir.dt.float32
BF16 = mybir.dt.bfloat16
EXPF = mybir.ActivationFunctionType.Exp
SCALE = 1.0 / np.sqrt(HD)


def _build_program():
    nc = bass.Bass("TRN2", target_bir_lowering=False, debug=False, num_devices=1)
    xT = nc.dram_tensor("xT", [D, S], BF16, kind="ExternalInput").ap()
    wq = nc.dram_tensor("wq", [D, FPC], BF16, kind="ExternalInput").ap()
    wk = nc.dram_tensor("wk", [D, FPC], BF16, kind="ExternalInput").ap()
    wv = nc.dram_tensor("wv", [D, FPC], BF16, kind="ExternalInput").ap()
    wo = nc.dram_tensor("wo", [FPC, D], BF16, kind="ExternalInput").ap()
    out = nc.dram_tensor("out", [S, D], F32, kind="ExternalOutput").ap()

    with tile.TileContext(nc) as tc:
        _emit(nc, tc, xT, wq, wk, wv, wo, out)
    _legalize_waits(nc)
    return nc


def _emit(nc, tc, xT, wq, wk, wv, wo, out):
    persist = tc.alloc_tile_pool(name="persist", bufs=1)
    dbl = tc.alloc_tile_pool(name="dbl", bufs=2)

    kT = persist.tile([P, NQB, S], BF16, tag="kT")
    vtm = persist.tile([P, NT, HPC, VW], BF16, tag="vtm")
    wo_sb = persist.tile([P, FPC // P, D], BF16, tag="wo_sb")
    wq_sb = persist.tile([P, DC, FPC], BF16, tag="wq_sb")
    wk_sb = persist.tile([P, DC, FPC], BF16, tag="wk_sb")
    wv_sb = persist.tile([P, DC, FPC], BF16, tag="wv_sb")
    dmask_f = persist.tile([P, P], F32, tag="dmask_f")
    dmask = persist.tile([P, P], BF16, tag="dmask")
    onesbf = persist.tile([P, HD], BF16, tag="onesbf")
    denst = persist.tile([P, QB], F32, tag="denst")
    recst = persist.tile([P, QB], F32, tag="recst")
    hilo = persist.tile([P, 2, QB], BF16, tag="hilo")
    denstP = persist.tile([P, QB], F32, tag="denstP")
    recstP = persist.tile([P, QB], F32, tag="recstP")
    hiloP = persist.tile([P, QB], BF16, tag="hiloP")

    # ---- input DMAs: wq + x(qb0) interleaved per dk chunk for fast lead-in
    xsb = [None, None]
    xsb[0] = dbl.tile([P, DC, QB], BF16, tag="xsb", name="xsb0")
    for dk in range(DC):
        nc.sync.dma_start(wq_sb[:, dk, :], wq[dk * P:(dk + 1) * P, :])
        nc.sync.dma_start(
            xsb[0][:, dk, :],
            xT[dk * P:(dk + 1) * P, 0:QB],
        )
    nc.sync.dma_start(wk_sb[:], wk.rearrange("(c p) e -> p c e", p=P))
    nc.sync.dma_start(wv_sb[:], wv.rearrange("(c p) e -> p c e", p=P))
    nc.sync.dma_start(wo_sb[:], wo.rearrange("(c p) e -> p c e", p=P))

    # one-time setup
    make_upper_triangular(nc, dmask_f[:], val=1.0, diag=True)
    nc.vector.tensor_copy(dmask[:], dmask_f[:])
    nc.vector.memset(onesbf[:], 1.0)
    nc.vector.memset(vtm[:, :, :, HD:HD + 1], 1.0)  # softmax-denominator ones

    with (
        tc.tile_pool(name="scps", bufs=1, space="PSUM") as scps,
        tc.tile_pool(name="ctxps", bufs=1, space="PSUM") as ctxps,
        tc.tile_pool(name="fill", bufs=2, space="PSUM") as fill,
        tc.tile_pool(name="esp", bufs=3) as esp,
        tc.tile_pool(name="dtmpp", bufs=4) as dtmpp,
        tc.tile_pool(name="stagep", bufs=3) as stagep,
    ):
        qcur = [None, None]
        ctxU = [None, None]
        ctxT = [None, None]

        # ---- emission helpers --------------------------------------------
        def proj_q(qb, m):
            ps = fill.tile([P, QB], F32, tag="fill")
            for dk in range(DC):
                nc.tensor.matmul(
                    ps[:],
                    lhsT=wq_sb[:, dk, m * P:(m + 1) * P],
                    rhs=xsb[qb % 2][:, dk, :],
                    start=(dk == 0),
                    stop=(dk == DC - 1),
                )
            nc.vector.tensor_copy(qcur[qb % 2][:, m, :], ps[:])

        def proj_k(qb, m):
            ps = fill.tile([P, QB], F32, tag="fill")
            for dk in range(DC):
                nc.tensor.matmul(
                    ps[:],
                    lhsT=wk_sb[:, dk, m * P:(m + 1) * P],
                    rhs=xsb[qb % 2][:, dk, :],
                    start=(dk == 0),
                    stop=(dk == DC - 1),
                )
            nc.vector.tensor_copy(kT[:, m, qb * QB:(qb + 1) * QB], ps[:])

        def proj_v(qb, i):
            nt = 4 * qb + i
            ps = fill.tile([P, FPC], F32, tag="fill")
            for dk in range(DC):
                nc.tensor.matmul(
                    ps[:],
                    lhsT=xsb[qb % 2][:, dk, i * P:(i + 1) * P],
                    rhs=wv_sb[:, dk, :],
                    start=(dk == 0),
                    stop=(dk == DC - 1),
                )
            nc.vector.tensor_copy(
                vtm[:, nt, :, 0:HD],
                ps[:].rearrange("p (h d) -> p h d", h=HPC),
            )

        # Diagonal block layout: (j, hh, block-slot, length). Each block
        # gets its OWN 512-wide PSUM bank: two row-tiled matmuls running
        # concurrently on PE row groups 0/64 wedge the PE if their outputs
        # share a PSUM bank (found empirically; the full groups are
        # naturally bank-separated).
        def diag_blocks(kind):
            j0 = 0 if kind == "d0" else 2
            return [(j0, 0, 0, QB - j0 * P), (j0, 1, 1, QB - j0 * P),
                    (j0 + 1, 0, 2, QB - (j0 + 1) * P),
                    (j0 + 1, 1, 3, QB - (j0 + 1) * P)]

        def emit_group(qb, m2, kind, kts):
            sc = scps.tile([P, 4, QB], F32, tag="sc")
            es = esp.tile([P, 4, QB], BF16, tag="es")
            q = qcur[qb % 2]
            if kind == "full":
                for i, kt in enumerate(kts):
                    for hh in range(2):
                        nc.tensor.matmul(
                            sc[:, i * 2 + hh, :],
                            lhsT=kT[hh * HD:(hh + 1) * HD, m2,
                                    kt * P:(kt + 1) * P],
                            rhs=q[hh * HD:(hh + 1) * HD, m2, :],
                            start=True,
                            stop=True,
                        )
                nb = len(kts) * 2
                nc.scalar.activation(es[:, 0:nb, :], sc[:, 0:nb, :], EXPF,
                                     scale=SCALE)
            else:
                blocks = diag_blocks(kind)
                for j, hh, b, ln in blocks:
                    nc.tensor.matmul(
                        sc[:, b, 0:ln],
                        lhsT=kT[hh * HD:(hh + 1) * HD, m2,
                                (4 * qb + j) * P:(4 * qb + j + 1) * P],
                        rhs=q[hh * HD:(hh + 1) * HD, m2, j * P:QB],
                        start=True,
                        stop=True,
                        skip_group_check=True,
                    )
                l01, l23 = blocks[0][3], blocks[2][3]
                nc.scalar.activation(es[:, 0:2, 0:l01], sc[:, 0:2, 0:l01],
                                     EXPF, scale=SCALE)
                nc.scalar.activation(es[:, 2:4, 0:l23], sc[:, 2:4, 0:l23],
                                     EXPF, scale=SCALE)
                for j, hh, b, ln in blocks:
                    nc.gpsimd.tensor_mul(
                        es[:, b, 0:P], es[:, b, 0:P], dmask[:]
                    )
            return es

        def make_ctx(qb, m2, kind, kts, es, pctx, first, last):
            def emit():
                if kind == "full":
                    for i, kt in enumerate(kts):
                        for hh in range(2):
                            nc.tensor.matmul(
                                pctx[hh][0:HD + 1, :],
                                lhsT=vtm[:, kt, 2 * m2 + hh, 0:HD + 1],
                                rhs=es[:, i * 2 + hh, :],
                                start=first[hh],
                                stop=False,
                                skip_group_check=True,
                            )
                            first[hh] = False
                else:
                    blocks = diag_blocks(kind)
                    for j, hh, b, ln in blocks:
                        nc.tensor.matmul(
                            pctx[hh][0:HD + 1, j * P:QB],
                            lhsT=vtm[:, 4 * qb + j, 2 * m2 + hh, 0:HD + 1],
                            rhs=es[:, b, 0:ln],
                            start=first[hh],
                            stop=(kind == "d1" and j == 3),
                            skip_group_check=True,
                        )
                        first[hh] = False
                if not last:
                    return
                for hh in range(2):
                    nc.vector.tensor_copy(
                        ctxU[qb % 2][hh * HD:(hh + 1) * HD, m2, :],
                        pctx[hh][0:HD, :],
                    )
                    dtmp = dtmpp.tile([P, QB], F32, tag="dtmp")
                    nc.vector.tensor_copy(
                        dtmp[HD:HD + 1, :], pctx[hh][HD:HD + 1, :]
                    )
                    if qb == NQB - 1:
                        idx = m2 * 32 + hh
                        nc.sync.dma_start(
                            denstP[idx:idx + 1, :], dtmp[HD:HD + 1, :]
                        )
                    else:
                        idx = qb * 32 + 2 * m2 + hh
                        nc.sync.dma_start(
                            denst[idx:idx + 1, :], dtmp[HD:HD + 1, :]
                        )
            return emit

        pending = [None]

        def fire():
            if pending[0] is not None:
                pending[0]()
                pending[0] = None

        # ---- main fused loop ---------------------------------------------
        # Software pipeline at qb granularity: qb's attention runs while
        # qb-1's epilogue chain (den -> recip -> broadcast -> normalize)
        # and out-projection interleave as PE filler, and qb+1's
        # q-projection is emitted before qb's epilogue so the PE queue
        # never stalls on the cross-engine chain.
        rrows = [None, None]

        def emit_epi_chain(qb, m2):
            # DVE/DMA-only part: reciprocal, bf16 row, ship to partition HD
            rr = slice(m2 * 32, m2 * 32 + 2)
            rrow = rrows[qb % 2]
            nc.vector.reciprocal(recstP[rr, :], denstP[rr, :])
            nc.vector.tensor_copy(hiloP[rr, :], recstP[rr, :])
            nc.sync.dma_start(rrow[HD:HD + 1, 2 * m2:2 * m2 + 2, :],
                              hiloP[rr, :])

        def emit_epi_bc(qb, m2):
            # PE broadcast (K=1 outer product) + normalize into ctxT
            rrow = rrows[qb % 2]
            bc = fill.tile([P, QB], F32, tag="fill")
            for half in range(2):
                nc.tensor.matmul(
                    bc[half * HD:(half + 1) * HD, :],
                    lhsT=onesbf[HD:HD + 1, :],
                    rhs=rrow[HD:HD + 1, 2 * m2 + half, :],
                    start=True,
                    stop=True,
                )
            nc.vector.tensor_mul(
                ctxT[qb % 2][:, m2, :], ctxU[qb % 2][:, m2, :], bc[:]
            )

        def emit_epilogue(qb):
            r = slice(qb * 32, qb * 32 + HPC)
            rrow = rrows[qb % 2]
            nc.vector.reciprocal(recst[r, :], denst[r, :])
            nc.vector.tensor_copy(hilo[r, 0, :], recst[r, :])
            nc.sync.dma_start(rrow[HD:HD + 1, :, :], hilo[r, 0:1, :])
            for m2 in range(NQB):
                bc = fill.tile([P, QB], F32, tag="fill")
                for half in range(2):
                    nc.tensor.matmul(
                        bc[half * HD:(half + 1) * HD, :],
                        lhsT=onesbf[HD:HD + 1, :],
                        rhs=rrow[HD:HD + 1, 2 * m2 + half, :],
                        start=True,
                        stop=True,
                    )
                nc.vector.tensor_mul(
                    ctxT[qb % 2][:, m2, :], ctxU[qb % 2][:, m2, :], bc[:]
                )

        def make_outproj(qb, i):
            def emit():
                nt = 4 * qb + i
                pso = [fill.tile([P, QB], F32, tag="fill",
                                 name=f"o{nt}_{ec}") for ec in range(2)]
                for m in range(FPC // P):
                    for ec in range(2):
                        nc.tensor.matmul(
                            pso[ec][:],
                            lhsT=ctxT[qb % 2][:, m, i * P:(i + 1) * P],
                            rhs=wo_sb[:, m, ec * QB:(ec + 1) * QB],
                            start=(m == 0),
                            stop=(m == FPC // P - 1),
                        )
                stage = stagep.tile([P, D], F32, tag="stage")
                nc.vector.tensor_copy(stage[:, 0:QB], pso[0][:])
                nc.scalar.copy(stage[:, QB:D], pso[1][:])
                nc.sync.dma_start(out[nt * P:(nt + 1) * P, :], stage[:])
            return emit

        qcur[0] = dbl.tile([P, NQB, QB], BF16, tag="qcur", name="qcur0")
        for m in range(NQB):
            proj_q(0, m)
        xsb[1] = dbl.tile([P, DC, QB], BF16, tag="xsb", name="xsb1")
        nc.sync.dma_start(
            xsb[1][:],
            xT[:, QB:2 * QB].rearrange("(c p) s -> p c s", p=P),
        )

        op_prev = []
        for qb in range(NQB):
            ctxU[qb % 2] = dbl.tile([P, NQB, QB], F32, tag="ctxU",
                                    name=f"ctxU{qb}")
            ctxT[qb % 2] = dbl.tile([P, NQB, QB], BF16, tag="ctxT",
                                    name=f"ctxT{qb}")
            rrows[qb % 2] = dbl.tile([P, HPC, QB], BF16, tag="rrow",
                                     name=f"rrow{qb}")

            # k/v filler units for this qb, interleaved into pair-0's full
            # groups (the diagonal is the only consumer of qb's own k/v).
            fillers = [lambda m=m: proj_k(qb, m) for m in range(NQB)]
            fillers += [lambda i=i: proj_v(qb, i) for i in range(4)]

            lastqb = (qb == NQB - 1)
            for m2 in range(NQB):
                if lastqb and m2 >= 1:
                    fire()
                    emit_epi_chain(qb, m2 - 1)
                    if m2 >= 2:
                        emit_epi_bc(qb, m2 - 2)
                pctx = {
                    hh: ctxps.tile([P, QB], F32, tag=f"pctx{hh}",
                                   name=f"pctx{hh}")
                    for hh in range(2)
                }
                first = {0: True, 1: True}
                groups = [("full", [kt, kt + 1])
                          for kt in range(0, 4 * qb, 2)]
                groups += [("d0", None), ("d1", None)]
                nfull = len(groups) - 2
                for gi, (kind, kts) in enumerate(groups):
                    if m2 == 0 and gi == nfull:
                        for f in fillers:
                            f()
                        fillers = []
                    es = emit_group(qb, m2, kind, kts)
                    fire()
                    pending[0] = make_ctx(qb, m2, kind, kts, es, pctx,
                                          first, gi == len(groups) - 1)
                    if m2 == 0 and fillers and gi < nfull:
                        take = max(1, (8 + nfull - 1) // max(nfull, 1) // 2)
                        for _ in range(min(take, len(fillers))):
                            fillers.pop(0)()
                if m2 >= 1 and op_prev:
                    op_prev.pop(0)()

            fire()
            for u in op_prev:
                u()
            op_prev = []

            if qb + 1 < NQB:
                qcur[(qb + 1) % 2] = dbl.tile([P, NQB, QB], BF16,
                                              tag="qcur",
                                              name=f"qcur{qb + 1}")
                for m in range(NQB):
                    proj_q(qb + 1, m)
                if qb + 2 < NQB:
                    xsb[(qb + 2) % 2] = dbl.tile([P, DC, QB], BF16,
                                                 tag="xsb",
                                                 name=f"xsb{qb + 2}")
                    nc.sync.dma_start(
                        xsb[(qb + 2) % 2][:],
                        xT[:, (qb + 2) * QB:(qb + 3) * QB].rearrange(
                            "(c p) s -> p c s", p=P
                        ),
                    )
                emit_epilogue(qb)
                op_prev = [make_outproj(qb, i) for i in range(4)]
            else:
                emit_epi_chain(qb, NQB - 1)
                emit_epi_bc(qb, NQB - 2)
                emit_epi_bc(qb, NQB - 1)
                for i in range(4):
                    make_outproj(qb, i)()

    dbl.release()
    persist.release()


_program_cache = None
last_results = None


def _get_program():
    global _program_cache
    if _program_cache is None:
        _program_cache = _build_program()
    return _program_cache


def kernel(x, Wq, Wk, Wv, Wo, bo):
    global last_results
    x = np.asarray(x, dtype=np.float32)
    Wq = np.asarray(Wq, dtype=np.float32)
    Wk = np.asarray(Wk, dtype=np.float32)
    Wv = np.asarray(Wv, dtype=np.float32)
    Wo = np.asarray(Wo, dtype=np.float32)
    bo = np.asarray(bo, dtype=np.float32)

    bf = ml_dtypes.bfloat16
    in_maps = []
    for c in range(NCORES):
        b, g = c // GPC, c % GPC
        fs = slice(g * FPC, (g + 1) * FPC)
        in_maps.append(
            {
                "xT": np.ascontiguousarray(x[b].T).astype(bf),
                "wq": np.ascontiguousarray(Wq[fs, :].T).astype(bf),
                "wk": np.ascontiguousarray(Wk[fs, :].T).astype(bf),
                "wv": np.ascontiguousarray(Wv[fs, :].T).astype(bf),
                "wo": np.ascontiguousarray(Wo[:, fs].T).astype(bf),
            }
        )

    nc = _get_program()
    res = run_bass_kernel_spmd(nc, in_maps, core_ids=list(range(NCORES)))
    last_results = res

    outf = np.empty((B, S, D), dtype=np.float32)
    for b in range(B):
        outf[b] = res.results[GPC * b]["out"] + res.results[GPC * b + 1]["out"] + bo
    return outf


# revision 17
# speedup vs baseline: 1.0987x; 1.0987x over previous
"""Causal multi-head attention (B=4, S=2048, D=1024, H=16) on 8 NeuronCores.

Sharding: core c handles batch b=c//2 and head-group g=c%2 (8 heads, 512
features). Host pre-transposes x and the weight slices (all bf16) so every
device matmul contracts along the partition dim; the row-parallel
out-projection partials are summed pairwise on the host (+ bias).

Per-core pipeline (one Bass/Tile program, SPMD over 8 cores), fused at
q-block (512-token) granularity so TensorE never drains while the ACT
engine chews softmax exps:

  for qb in 0..3:
    1. q-projection for qb's tokens (bf16 matmuls, 8 dk chunks per m-tile).
    2. causal attention for qb against k-tiles 0..4qb+3, two heads per
       pair-stream packed at PE row offsets 0/64 so their K=64 score
       matmuls run concurrently in the array; one combined 2-head scores
       PSUM tile + a single exp per group keeps the pair adjacent in the
       PE queue. k/v-projections for qb's own tokens are emitted between
       pair-0's full groups as PE filler (only the diagonal needs them).
    3. denominators from an appended ones-column in the 65-wide V
       stationary; batched approx reciprocal, hi/lo bf16 split broadcast
       with K=2 PE outer products; normalize into bf16 ctxT.
    4. out-projection of qb's 4 token tiles; PSUM evicted by the ACT
       engine; f32 DMA out.
"""

import sys
import types

import numpy as np
import ml_dtypes

import concourse.bass as bass
import concourse.mybir as mybir
from concourse import tile
from concourse.bass_utils import run_bass_kernel_spmd
from concourse.masks import make_upper_triangular

# ----------------------------------------------------------------------------
# Compat patches for this container (self-contained on purpose).
# ----------------------------------------------------------------------------


def _patch_tail_drain():
    """This walrus build accepts only ONE sync-wait per sync-engine
    instruction; TileContext's tail drain may carry several. Split extras
    onto dedicated 1-wait nops."""
    from concourse.vector_clock import ScopedClock

    def _drain_and_barrier(self, tick_clock, wait_clock):
        nc = self.nc
        drain_inst = nc.sync.drain()
        wait_clock.add_sem_waits(
            drain_inst.ins, ScopedClock({None: tick_clock.global_clock})
        )
        si = drain_inst.ins.sync_info
        if si is not None and len(si.on_wait) > 1:
            waits = list(si.on_wait)
            drain_inst.ins.sync_info = mybir.SyncInfo(
                on_wait=waits[:1], on_update=list(si.on_update)
            )
            for w in waits[1:]:
                n = nc.sync.nop()
                n.ins.sync_info = mybir.SyncInfo(on_wait=[w], on_update=[])

        nc.all_engine_barrier()
        assert self.sems is not None
        popped = nc._tile_sem_poison_stack.pop()
        assert popped is self._sem_poison
        nc.clear_and_free_semaphores(list(self.sems.allocated().values()))
        nc.all_engine_barrier()

    tile.TileContext._drain_and_barrier = _drain_and_barrier


def _patch_profiling():
    """Provide the NTFF profile hook (image's antenv lacks axon_hooks) and
    disable cloud artifact uploads. Only matters when tracing is requested."""
    import concourse.bass_utils as bass_utils

    bass_utils.upload_artifacts = lambda tmpdir: tmpdir
    try:
        from antenv.axon_hooks import get_axon_ntff_profile_hook  # noqa: F401
        return
    except ImportError:
        pass
    try:
        from trn_agent_boot.trn_boot import _ntff_profile_via_ctypes

        hook = _ntff_profile_via_ctypes("/opt/axon/libaxon_pjrt.so")
    except Exception:
        hook = None
    mod = types.ModuleType("antenv.axon_hooks")
    mod._hook = hook
    mod.get_axon_ntff_profile_hook = lambda: mod._hook
    mod.set_axon_ntff_profile_hook = lambda h: setattr(mod, "_hook", h)
    sys.modules["antenv.axon_hooks"] = mod
    import antenv

    antenv.axon_hooks = mod


_patch_tail_drain()
_patch_profiling()


def _legalize_waits(nc):
    """This walrus build allows 1 sync-wait per instruction (2 on
    EventSemaphore). Split excess waits onto EventSemaphore carriers
    inserted just before the over-capacity instruction (same engine
    queue, so ordering semantics are preserved)."""
    n_fix = 0
    for f in nc.m.functions:
        for b in f.blocks:
            out = []
            changed = False
            for inst in b.instructions:
                si = inst.sync_info
                cap = 1
                if si is not None and len(si.on_wait) > cap:
                    waits = list(si.on_wait)
                    extra, keep = waits[:-cap], waits[-cap:]
                    for i in range(0, len(extra), 1):
                        n_fix += 1
                        out.append(
                            mybir.InstNoOp(
                                name=f"I-waitfix-{n_fix}",
                                engine=inst.engine,
                                ins=[],
                                outs=[],
                                sync_info=mybir.SyncInfo(
                                    on_wait=extra[i:i + 1], on_update=[]
                                ),
                            )
                        )
                    inst.sync_info = mybir.SyncInfo(
                        on_wait=keep, on_update=list(si.on_update)
                    )
                    changed = True
                out.append(inst)
            if changed:
                b.instructions = out

# ----------------------------------------------------------------------------
# Problem constants (hardcoded; kernel.py must be self-contained).
# ----------------------------------------------------------------------------
B, S, D, H = 4, 2048, 1024, 16
HD = D // H          # 64 head dim
NCORES = 8
GPC = 2              # head-groups per batch (cores per batch)
FPC = D // GPC       # 512 features per core
HPC = H // GPC       # 8 heads per core
P = 128
DC = D // P          # 8 contraction chunks
NT = S // P          # 16 token tiles
QB = 512             # q-block
NQB = S // QB        # 4
VW = 66              # vtm row width: 64 v dims + ones col + pad

F32 = mybir.dt.float32
BF16 = mybir.dt.bfloat16
EXPF = mybir.ActivationFunctionType.Exp
SCALE = 1.0 / np.sqrt(HD)


def _build_program():
    nc = bass.Bass("TRN2", target_bir_lowering=False, debug=False, num_devices=1)
    xT = nc.dram_tensor("xT", [D, S], BF16, kind="ExternalInput").ap()
    wq = nc.dram_tensor("wq", [D, FPC], BF16, kind="ExternalInput").ap()
    wk = nc.dram_tensor("wk", [D, FPC], BF16, kind="ExternalInput").ap()
    wv = nc.dram_tensor("wv", [D, FPC], BF16, kind="ExternalInput").ap()
    wo = nc.dram_tensor("wo", [FPC, D], BF16, kind="ExternalInput").ap()
    out = nc.dram_tensor("out", [S, D], F32, kind="ExternalOutput").ap()

    with tile.TileContext(nc) as tc:
        _emit(nc, tc, xT, wq, wk, wv, wo, out)
    _legalize_waits(nc)
    return nc


def _emit(nc, tc, xT, wq, wk, wv, wo, out):
    persist = tc.alloc_tile_pool(name="persist", bufs=1)
    dbl = tc.alloc_tile_pool(name="dbl", bufs=2)

    kT = persist.tile([P, NQB, S], BF16, tag="kT")
    vtm = persist.tile([P, NT, HPC, VW], BF16, tag="vtm")
    wo_sb = persist.tile([P, FPC // P, D], BF16, tag="wo_sb")
    wq_sb = persist.tile([P, DC, FPC], BF16, tag="wq_sb")
    wk_sb = persist.tile([P, DC, FPC], BF16, tag="wk_sb")
    wv_sb = persist.tile([P, DC, FPC], BF16, tag="wv_sb")
    dmask_f = persist.tile([P, P], F32, tag="dmask_f")
    dmask = persist.tile([P, P], BF16, tag="dmask")
    onesbf = persist.tile([P, HD], BF16, tag="onesbf")
    denst = persist.tile([P, QB], F32, tag="denst")
    recst = persist.tile([P, QB], F32, tag="recst")
    hilo = persist.tile([P, 2, QB], BF16, tag="hilo")
    denstP = persist.tile([P, QB], F32, tag="denstP")
    recstP = persist.tile([P, QB], F32, tag="recstP")
    hiloP = persist.tile([P, QB], BF16, tag="hiloP")

    # ---- input DMAs: wq + x(qb0) interleaved per dk chunk for fast lead-in
    xsb = [None, None]
    xsb[0] = dbl.tile([P, DC, QB], BF16, tag="xsb", name="xsb0")
    for dk in range(DC):
        nc.sync.dma_start(wq_sb[:, dk, :], wq[dk * P:(dk + 1) * P, :])
        nc.sync.dma_start(
            xsb[0][:, dk, :],
            xT[dk * P:(dk + 1) * P, 0:QB],
        )
    nc.sync.dma_start(wk_sb[:], wk.rearrange("(c p) e -> p c e", p=P))
    nc.sync.dma_start(wv_sb[:], wv.rearrange("(c p) e -> p c e", p=P))
    nc.sync.dma_start(wo_sb[:], wo.rearrange("(c p) e -> p c e", p=P))

    # one-time setup
    make_upper_triangular(nc, dmask_f[:], val=1.0, diag=True)
    nc.vector.tensor_copy(dmask[:], dmask_f[:])
    nc.vector.memset(onesbf[:], 1.0)
    nc.vector.memset(vtm[:, :, :, HD:HD + 1], 1.0)  # softmax-denominator ones

    with (
        tc.tile_pool(name="scps", bufs=1, space="PSUM") as scps,
        tc.tile_pool(name="ctxps", bufs=1, space="PSUM") as ctxps,
        tc.tile_pool(name="fill", bufs=2, space="PSUM") as fill,
        tc.tile_pool(name="esp", bufs=3) as esp,
        tc.tile_pool(name="dtmpp", bufs=4) as dtmpp,
        tc.tile_pool(name="stagep", bufs=3) as stagep,
    ):
        qcur = [None, None]
        ctxU = [None, None]
        ctxT = [None, None]

        # ---- emission helpers --------------------------------------------
        def proj_q(qb, m):
            ps = fill.tile([P, QB], F32, tag="fill")
            for dk in range(DC):
                nc.tensor.matmul(
                    ps[:],
                    lhsT=wq_sb[:, dk, m * P:(m + 1) * P],
                    rhs=xsb[qb % 2][:, dk, :],
                    start=(dk == 0),
                    stop=(dk == DC - 1),
                )
            nc.vector.tensor_copy(qcur[qb % 2][:, m, :], ps[:])

        def proj_k(qb, m):
            ps = fill.tile([P, QB], F32, tag="fill")
            for dk in range(DC):
                nc.tensor.matmul(
                    ps[:],
                    lhsT=wk_sb[:, dk, m * P:(m + 1) * P],
                    rhs=xsb[qb % 2][:, dk, :],
                    start=(dk == 0),
                    stop=(dk == DC - 1),
                )
            nc.vector.tensor_copy(kT[:, m, qb * QB:(qb + 1) * QB], ps[:])

        def proj_v(qb, i):
            nt = 4 * qb + i
            ps = fill.tile([P, FPC], F32, tag="fill")
            for dk in range(DC):
                nc.tensor.matmul(
                    ps[:],
                    lhsT=xsb[qb % 2][:, dk, i * P:(i + 1) * P],
                    rhs=wv_sb[:, dk, :],
                    start=(dk == 0),
                    stop=(dk == DC - 1),
                )
            nc.vector.tensor_copy(
                vtm[:, nt, :, 0:HD],
                ps[:].rearrange("p (h d) -> p h d", h=HPC),
            )

        # Diagonal block layout: (j, hh, block-slot, length). Each block
        # gets its OWN 512-wide PSUM bank: two row-tiled matmuls running
        # concurrently on PE row groups 0/64 wedge the PE if their outputs
        # share a PSUM bank (found empirically; the full groups are
        # naturally bank-separated).
        def diag_blocks(kind):
            j0 = 0 if kind == "d0" else 2
            return [(j0, 0, 0, QB - j0 * P), (j0, 1, 1, QB - j0 * P),
                    (j0 + 1, 0, 2, QB - (j0 + 1) * P),
                    (j0 + 1, 1, 3, QB - (j0 + 1) * P)]

        def emit_group(qb, m2, kind, kts):
            sc = scps.tile([P, 4, QB], F32, tag="sc")
            es = esp.tile([P, 4, QB], BF16, tag="es")
            q = qcur[qb % 2]
            if kind == "full":
                for i, kt in enumerate(kts):
                    for hh in range(2):
                        nc.tensor.matmul(
                            sc[:, i * 2 + hh, :],
                            lhsT=kT[hh * HD:(hh + 1) * HD, m2,
                                    kt * P:(kt + 1) * P],
                            rhs=q[hh * HD:(hh + 1) * HD, m2, :],
                            start=True,
                            stop=True,
                        )
                nb = len(kts) * 2
                nc.scalar.activation(es[:, 0:nb, :], sc[:, 0:nb, :], EXPF,
                                     scale=SCALE)
            else:
                blocks = diag_blocks(kind)
                for j, hh, b, ln in blocks:
                    nc.tensor.matmul(
                        sc[:, b, 0:ln],
                        lhsT=kT[hh * HD:(hh + 1) * HD, m2,
                                (4 * qb + j) * P:(4 * qb + j + 1) * P],
                        rhs=q[hh * HD:(hh + 1) * HD, m2, j * P:QB],
                        start=True,
                        stop=True,
                        skip_group_check=True,
                    )
                l01, l23 = blocks[0][3], blocks[2][3]
                nc.scalar.activation(es[:, 0:2, 0:l01], sc[:, 0:2, 0:l01],
                                     EXPF, scale=SCALE)
                nc.scalar.activation(es[:, 2:4, 0:l23], sc[:, 2:4, 0:l23],
                                     EXPF, scale=SCALE)
                for j, hh, b, ln in blocks:
                    nc.gpsimd.tensor_mul(
                        es[:, b, 0:P], es[:, b, 0:P], dmask[:]
                    )
            return es

        def make_ctx(qb, m2, kind, kts, es, pctx, first, last):
            def emit():
                if kind == "full":
                    for i, kt in enumerate(kts):
                        for hh in range(2):
                            nc.tensor.matmul(
                                pctx[hh][0:HD + 1, :],
                                lhsT=vtm[:, kt, 2 * m2 + hh, 0:HD + 1],
                                rhs=es[:, i * 2 + hh, :],
                                start=first[hh],
                                stop=False,
                                skip_group_check=True,
                            )
                            first[hh] = False
                else:
                    blocks = diag_blocks(kind)
                    for j, hh, b, ln in blocks:
                        nc.tensor.matmul(
                            pctx[hh][0:HD + 1, j * P:QB],
                            lhsT=vtm[:, 4 * qb + j, 2 * m2 + hh, 0:HD + 1],
                            rhs=es[:, b, 0:ln],
                            start=first[hh],
                            stop=(kind == "d1" and j == 3),
                            skip_group_check=True,
                        )
                        first[hh] = False
                if not last:
                    return
                for hh in range(2):
                    eng = nc.vector if hh == 0 else nc.scalar
                    cp = (nc.vector.tensor_copy if hh == 0
                          else nc.scalar.copy)
                    cp(
                        ctxU[qb % 2][hh * HD:(hh + 1) * HD, m2, :],
                        pctx[hh][0:HD, :],
                    )
                    dtmp = dtmpp.tile([P, QB], F32, tag="dtmp")
                    cp(dtmp[HD:HD + 1, :], pctx[hh][HD:HD + 1, :])
                    if qb == NQB - 1:
                        idx = m2 * 32 + hh
                        nc.sync.dma_start(
                            denstP[idx:idx + 1, :], dtmp[HD:HD + 1, :]
                        )
                    else:
                        idx = qb * 32 + 2 * m2 + hh
                        nc.sync.dma_start(
                            denst[idx:idx + 1, :], dtmp[HD:HD + 1, :]
                        )
            return emit

        pending = [None]

        def fire():
            if pending[0] is not None:
                pending[0]()
                pending[0] = None

        # ---- main fused loop ---------------------------------------------
        # Software pipeline at qb granularity: qb's attention runs while
        # qb-1's epilogue chain (den -> recip -> broadcast -> normalize)
        # and out-projection interleave as PE filler, and qb+1's
        # q-projection is emitted before qb's epilogue so the PE queue
        # never stalls on the cross-engine chain.
        rrows = [None, None]

        def emit_epi_chain(qb, m2):
            # DVE/DMA-only part: reciprocal, bf16 row, ship to partition HD
            rr = slice(m2 * 32, m2 * 32 + 2)
            rrow = rrows[qb % 2]
            nc.vector.reciprocal(recstP[rr, :], denstP[rr, :])
            nc.vector.tensor_copy(hiloP[rr, :], recstP[rr, :])
            nc.sync.dma_start(rrow[HD:HD + 1, 2 * m2:2 * m2 + 2, :],
                              hiloP[rr, :])

        def emit_epi_bc(qb, m2):
            # PE broadcast (K=1 outer product) + normalize into ctxT
            rrow = rrows[qb % 2]
            bc = fill.tile([P, QB], F32, tag="fill")
            for half in range(2):
                nc.tensor.matmul(
                    bc[half * HD:(half + 1) * HD, :],
                    lhsT=onesbf[HD:HD + 1, :],
                    rhs=rrow[HD:HD + 1, 2 * m2 + half, :],
                    start=True,
                    stop=True,
                )
            nc.vector.tensor_mul(
                ctxT[qb % 2][:, m2, :], ctxU[qb % 2][:, m2, :], bc[:]
            )

        def emit_epilogue(qb):
            r = slice(qb * 32, qb * 32 + HPC)
            rrow = rrows[qb % 2]
            nc.vector.reciprocal(recst[r, :], denst[r, :])
            nc.vector.tensor_copy(hilo[r, 0, :], recst[r, :])
            nc.sync.dma_start(rrow[HD:HD + 1, :, :], hilo[r, 0:1, :])
            for m2 in range(NQB):
                bc = fill.tile([P, QB], F32, tag="fill")
                for half in range(2):
                    nc.tensor.matmul(
                        bc[half * HD:(half + 1) * HD, :],
                        lhsT=onesbf[HD:HD + 1, :],
                        rhs=rrow[HD:HD + 1, 2 * m2 + half, :],
                        start=True,
                        stop=True,
                    )
                nc.vector.tensor_mul(
                    ctxT[qb % 2][:, m2, :], ctxU[qb % 2][:, m2, :], bc[:]
                )

        def make_outproj(qb, i):
            def emit():
                nt = 4 * qb + i
                pso = [fill.tile([P, QB], F32, tag="fill",
                                 name=f"o{nt}_{ec}") for ec in range(2)]
                for m in range(FPC // P):
                    for ec in range(2):
                        nc.tensor.matmul(
                            pso[ec][:],
                            lhsT=ctxT[qb % 2][:, m, i * P:(i + 1) * P],
                            rhs=wo_sb[:, m, ec * QB:(ec + 1) * QB],
                            start=(m == 0),
                            stop=(m == FPC // P - 1),
                        )
                stage = stagep.tile([P, D], F32, tag="stage")
                nc.vector.tensor_copy(stage[:, 0:QB], pso[0][:])
                nc.scalar.copy(stage[:, QB:D], pso[1][:])
                nc.sync.dma_start(out[nt * P:(nt + 1) * P, :], stage[:])
            return emit

        qcur[0] = dbl.tile([P, NQB, QB], BF16, tag="qcur", name="qcur0")
        for m in range(NQB):
            proj_q(0, m)
        xsb[1] = dbl.tile([P, DC, QB], BF16, tag="xsb", name="xsb1")
        nc.sync.dma_start(
            xsb[1][:],
            xT[:, QB:2 * QB].rearrange("(c p) s -> p c s", p=P),
        )

        ep_prev = None
        op_prev = []
        for qb in range(NQB):
            ctxU[qb % 2] = dbl.tile([P, NQB, QB], F32, tag="ctxU",
                                    name=f"ctxU{qb}")
            ctxT[qb % 2] = dbl.tile([P, NQB, QB], BF16, tag="ctxT",
                                    name=f"ctxT{qb}")
            rrows[qb % 2] = dbl.tile([P, HPC, QB], BF16, tag="rrow",
                                     name=f"rrow{qb}")

            # k/v filler units for this qb, interleaved into pair-0's full
            # groups (the diagonal is the only consumer of qb's own k/v).
            fillers = [lambda m=m: proj_k(qb, m) for m in range(NQB)]
            fillers += [lambda i=i: proj_v(qb, i) for i in range(4)]

            lastqb = (qb == NQB - 1)
            for m2 in range(NQB):
                if lastqb and m2 >= 1:
                    fire()
                    emit_epi_chain(qb, m2 - 1)
                    if m2 >= 2:
                        emit_epi_bc(qb, m2 - 2)
                pctx = {
                    hh: ctxps.tile([P, QB], F32, tag=f"pctx{hh}",
                                   name=f"pctx{hh}")
                    for hh in range(2)
                }
                first = {0: True, 1: True}
                groups = [("full", [kt, kt + 1])
                          for kt in range(0, 4 * qb, 2)]
                groups += [("d0", None), ("d1", None)]
                nfull = len(groups) - 2
                for gi, (kind, kts) in enumerate(groups):
                    if m2 == 0 and gi == nfull:
                        for f in fillers:
                            f()
                        fillers = []
                    es = emit_group(qb, m2, kind, kts)
                    fire()
                    pending[0] = make_ctx(qb, m2, kind, kts, es, pctx,
                                          first, gi == len(groups) - 1)
                    if m2 == 0 and fillers and gi < nfull:
                        take = max(1, (8 + nfull - 1) // max(nfull, 1) // 2)
                        for _ in range(min(take, len(fillers))):
                            fillers.pop(0)()
                if m2 == 0 and ep_prev is not None:
                    ep_prev()
                    ep_prev = None
                if m2 >= 1 and op_prev:
                    op_prev.pop(0)()

            fire()
            for u in op_prev:
                u()
            op_prev = []

            if qb + 1 < NQB:
                qcur[(qb + 1) % 2] = dbl.tile([P, NQB, QB], BF16,
                                              tag="qcur",
                                              name=f"qcur{qb + 1}")
                for m in range(NQB):
                    proj_q(qb + 1, m)
                if qb + 2 < NQB:
                    xsb[(qb + 2) % 2] = dbl.tile([P, DC, QB], BF16,
                                                 tag="xsb",
                                                 name=f"xsb{qb + 2}")
                    nc.sync.dma_start(
                        xsb[(qb + 2) % 2][:],
                        xT[:, (qb + 2) * QB:(qb + 3) * QB].rearrange(
                            "(c p) s -> p c s", p=P
                        ),
                    )
                ep_prev = (lambda qb=qb: emit_epilogue(qb))
                op_prev = [make_outproj(qb, i) for i in range(4)]
            else:
                emit_epi_chain(qb, NQB - 1)
                emit_epi_bc(qb, NQB - 2)
                emit_epi_bc(qb, NQB - 1)
                for i in range(4):
                    make_outproj(qb, i)()

    dbl.release()
    persist.release()


_program_cache = None
last_results = None


def _get_program():
    global _program_cache
    if _program_cache is None:
        _program_cache = _build_program()
    return _program_cache


def kernel(x, Wq, Wk, Wv, Wo, bo):
    global last_results
    x = np.asarray(x, dtype=np.float32)
    Wq = np.asarray(Wq, dtype=np.float32)
    Wk = np.asarray(Wk, dtype=np.float32)
    Wv = np.asarray(Wv, dtype=np.float32)
    Wo = np.asarray(Wo, dtype=np.float32)
    bo = np.asarray(bo, dtype=np.float32)

    bf = ml_dtypes.bfloat16
    in_maps = []
    for c in range(NCORES):
        b, g = c // GPC, c % GPC
        fs = slice(g * FPC, (g + 1) * FPC)
        in_maps.append(
            {
                "xT": np.ascontiguousarray(x[b].T).astype(bf),
                "wq": np.ascontiguousarray(Wq[fs, :].T).astype(bf),
                "wk": np.ascontiguousarray(Wk[fs, :].T).astype(bf),
                "wv": np.ascontiguousarray(Wv[fs, :].T).astype(bf),
                "wo": np.ascontiguousarray(Wo[:, fs].T).astype(bf),
            }
        )

    nc = _get_program()
    res = run_bass_kernel_spmd(nc, in_maps, core_ids=list(range(NCORES)))
    last_results = res

    outf = np.empty((B, S, D), dtype=np.float32)
    for b in range(B):
        outf[b] = res.results[GPC * b]["out"] + res.results[GPC * b + 1]["out"] + bo
    return outf


# revision 19
# speedup vs baseline: 1.1285x; 1.0271x over previous
"""Causal multi-head attention (B=4, S=2048, D=1024, H=16) on 8 NeuronCores.

Sharding: core c handles batch b=c//2 and head-group g=c%2 (8 heads, 512
features). Host pre-transposes x and the weight slices (all bf16) so every
device matmul contracts along the partition dim; the row-parallel
out-projection partials are summed pairwise on the host (+ bias).

Per-core pipeline (one Bass/Tile program, SPMD over 8 cores), fused at
q-block (512-token) granularity so TensorE never drains while the ACT
engine chews softmax exps:

  for qb in 0..3:
    1. q-projection for qb's tokens (bf16 matmuls, 8 dk chunks per m-tile).
    2. causal attention for qb against k-tiles 0..4qb+3, two heads per
       pair-stream packed at PE row offsets 0/64 so their K=64 score
       matmuls run concurrently in the array; one combined 2-head scores
       PSUM tile + a single exp per group keeps the pair adjacent in the
       PE queue. k/v-projections for qb's own tokens are emitted between
       pair-0's full groups as PE filler (only the diagonal needs them).
    3. denominators from an appended ones-column in the 65-wide V
       stationary; batched approx reciprocal, hi/lo bf16 split broadcast
       with K=2 PE outer products; normalize into bf16 ctxT.
    4. out-projection of qb's 4 token tiles; PSUM evicted by the ACT
       engine; f32 DMA out.
"""

import sys
import types

import numpy as np
import ml_dtypes

import concourse.bass as bass
import concourse.mybir as mybir
from concourse import tile
from concourse.bass_utils import run_bass_kernel_spmd
from concourse.masks import make_upper_triangular

# ----------------------------------------------------------------------------
# Compat patches for this container (self-contained on purpose).
# ----------------------------------------------------------------------------


def _patch_tail_drain():
    """This walrus build accepts only ONE sync-wait per sync-engine
    instruction; TileContext's tail drain may carry several. Split extras
    onto dedicated 1-wait nops."""
    from concourse.vector_clock import ScopedClock

    def _drain_and_barrier(self, tick_clock, wait_clock):
        nc = self.nc
        drain_inst = nc.sync.drain()
        wait_clock.add_sem_waits(
            drain_inst.ins, ScopedClock({None: tick_clock.global_clock})
        )
        si = drain_inst.ins.sync_info
        if si is not None and len(si.on_wait) > 1:
            waits = list(si.on_wait)
            drain_inst.ins.sync_info = mybir.SyncInfo(
                on_wait=waits[:1], on_update=list(si.on_update)
            )
            for w in waits[1:]:
                n = nc.sync.nop()
                n.ins.sync_info = mybir.SyncInfo(on_wait=[w], on_update=[])

        nc.all_engine_barrier()
        assert self.sems is not None
        popped = nc._tile_sem_poison_stack.pop()
        assert popped is self._sem_poison
        nc.clear_and_free_semaphores(list(self.sems.allocated().values()))
        nc.all_engine_barrier()

    tile.TileContext._drain_and_barrier = _drain_and_barrier


def _patch_profiling():
    """Provide the NTFF profile hook (image's antenv lacks axon_hooks) and
    disable cloud artifact uploads. Only matters when tracing is requested."""
    import concourse.bass_utils as bass_utils

    bass_utils.upload_artifacts = lambda tmpdir: tmpdir
    try:
        from antenv.axon_hooks import get_axon_ntff_profile_hook  # noqa: F401
        return
    except ImportError:
        pass
    try:
        from trn_agent_boot.trn_boot import _ntff_profile_via_ctypes

        hook = _ntff_profile_via_ctypes("/opt/axon/libaxon_pjrt.so")
    except Exception:
        hook = None
    mod = types.ModuleType("antenv.axon_hooks")
    mod._hook = hook
    mod.get_axon_ntff_profile_hook = lambda: mod._hook
    mod.set_axon_ntff_profile_hook = lambda h: setattr(mod, "_hook", h)
    sys.modules["antenv.axon_hooks"] = mod
    import antenv

    antenv.axon_hooks = mod


_patch_tail_drain()
_patch_profiling()


def _legalize_waits(nc):
    """This walrus build allows 1 sync-wait per instruction (2 on
    EventSemaphore). Split excess waits onto EventSemaphore carriers
    inserted just before the over-capacity instruction (same engine
    queue, so ordering semantics are preserved)."""
    n_fix = 0
    for f in nc.m.functions:
        for b in f.blocks:
            out = []
            changed = False
            for inst in b.instructions:
                si = inst.sync_info
                cap = 1
                if si is not None and len(si.on_wait) > cap:
                    waits = list(si.on_wait)
                    extra, keep = waits[:-cap], waits[-cap:]
                    for i in range(0, len(extra), 1):
                        n_fix += 1
                        out.append(
                            mybir.InstNoOp(
                                name=f"I-waitfix-{n_fix}",
                                engine=inst.engine,
                                ins=[],
                                outs=[],
                                sync_info=mybir.SyncInfo(
                                    on_wait=extra[i:i + 1], on_update=[]
                                ),
                            )
                        )
                    inst.sync_info = mybir.SyncInfo(
                        on_wait=keep, on_update=list(si.on_update)
                    )
                    changed = True
                out.append(inst)
            if changed:
                b.instructions = out

# ----------------------------------------------------------------------------
# Problem constants (hardcoded; kernel.py must be self-contained).
# ----------------------------------------------------------------------------
B, S, D, H = 4, 2048, 1024, 16
HD = D // H          # 64 head dim
NCORES = 8
GPC = 2              # head-groups per batch (cores per batch)
FPC = D // GPC       # 512 features per core
HPC = H // GPC       # 8 heads per core
P = 128
DC = D // P          # 8 contraction chunks
NT = S // P          # 16 token tiles
QB = 512             # q-block
NQB = S // QB        # 4
VW = 66              # vtm row width: 64 v dims + ones col + pad

F32 = mybir.dt.float32
BF16 = mybir.dt.bfloat16
EXPF = mybir.ActivationFunctionType.Exp
SCALE = 1.0 / np.sqrt(HD)


def _build_program():
    nc = bass.Bass("TRN2", target_bir_lowering=False, debug=False, num_devices=1)
    xT = nc.dram_tensor("xT", [D, S], BF16, kind="ExternalInput").ap()
    wq = nc.dram_tensor("wq", [D, FPC], BF16, kind="ExternalInput").ap()
    wk = nc.dram_tensor("wk", [D, FPC], BF16, kind="ExternalInput").ap()
    wv = nc.dram_tensor("wv", [D, FPC], BF16, kind="ExternalInput").ap()
    wo = nc.dram_tensor("wo", [FPC, D], BF16, kind="ExternalInput").ap()
    out = nc.dram_tensor("out", [S, D], F32, kind="ExternalOutput").ap()

    with tile.TileContext(nc) as tc:
        _emit(nc, tc, xT, wq, wk, wv, wo, out)
    _legalize_waits(nc)
    return nc


def _emit(nc, tc, xT, wq, wk, wv, wo, out):
    persist = tc.alloc_tile_pool(name="persist", bufs=1)
    dbl = tc.alloc_tile_pool(name="dbl", bufs=2)

    kT = persist.tile([P, NQB, S], BF16, tag="kT")
    vtm = persist.tile([P, NT, HPC, VW], BF16, tag="vtm")
    wo_sb = persist.tile([P, FPC // P, D], BF16, tag="wo_sb")
    wq_sb = persist.tile([P, DC, FPC], BF16, tag="wq_sb")
    wk_sb = persist.tile([P, DC, FPC], BF16, tag="wk_sb")
    wv_sb = persist.tile([P, DC, FPC], BF16, tag="wv_sb")
    dmask_f = persist.tile([P, P], F32, tag="dmask_f")
    dmask = persist.tile([P, P], BF16, tag="dmask")
    onesbf = persist.tile([P, HD], BF16, tag="onesbf")
    denst = persist.tile([P, QB], F32, tag="denst")
    recst = persist.tile([P, QB], F32, tag="recst")
    hilo = persist.tile([P, 2, QB], BF16, tag="hilo")
    denstP = persist.tile([P, QB], F32, tag="denstP")
    recstP = persist.tile([P, QB], F32, tag="recstP")
    hiloP = persist.tile([P, QB], BF16, tag="hiloP")

    # ---- input DMAs: wq + x(qb0) interleaved per dk chunk for fast lead-in
    xsb = [None, None]
    xsb[0] = dbl.tile([P, DC, QB], BF16, tag="xsb", name="xsb0")
    for dk in range(DC):
        nc.sync.dma_start(wq_sb[:, dk, :], wq[dk * P:(dk + 1) * P, :])
        nc.sync.dma_start(
            xsb[0][:, dk, :],
            xT[dk * P:(dk + 1) * P, 0:QB],
        )
    nc.sync.dma_start(wk_sb[:], wk.rearrange("(c p) e -> p c e", p=P))
    nc.sync.dma_start(wv_sb[:], wv.rearrange("(c p) e -> p c e", p=P))
    nc.sync.dma_start(wo_sb[:], wo.rearrange("(c p) e -> p c e", p=P))

    # one-time setup
    make_upper_triangular(nc, dmask_f[:], val=1.0, diag=True)
    nc.vector.tensor_copy(dmask[:], dmask_f[:])
    nc.vector.memset(onesbf[:], 1.0)
    nc.vector.memset(vtm[:, :, :, HD:HD + 1], 1.0)  # softmax-denominator ones

    with (
        tc.tile_pool(name="scps", bufs=2, space="PSUM") as scps,
        tc.tile_pool(name="ctxps", bufs=1, space="PSUM") as ctxps,
        tc.tile_pool(name="fill", bufs=2, space="PSUM") as fill,
        tc.tile_pool(name="esp", bufs=4) as esp,
        tc.tile_pool(name="dtmpp", bufs=4) as dtmpp,
        tc.tile_pool(name="stagep", bufs=3) as stagep,
    ):
        qcur = [None, None]
        ctxU = [None, None]
        ctxT = [None, None]

        # ---- emission helpers --------------------------------------------
        def proj_q(qb, m):
            ps = fill.tile([P, QB], F32, tag="fill")
            for dk in range(DC):
                nc.tensor.matmul(
                    ps[:],
                    lhsT=wq_sb[:, dk, m * P:(m + 1) * P],
                    rhs=xsb[qb % 2][:, dk, :],
                    start=(dk == 0),
                    stop=(dk == DC - 1),
                )
            nc.vector.tensor_copy(qcur[qb % 2][:, m, :], ps[:])

        def proj_k(qb, m):
            ps = fill.tile([P, QB], F32, tag="fill")
            for dk in range(DC):
                nc.tensor.matmul(
                    ps[:],
                    lhsT=wk_sb[:, dk, m * P:(m + 1) * P],
                    rhs=xsb[qb % 2][:, dk, :],
                    start=(dk == 0),
                    stop=(dk == DC - 1),
                )
            nc.vector.tensor_copy(kT[:, m, qb * QB:(qb + 1) * QB], ps[:])

        def proj_v(qb, i):
            nt = 4 * qb + i
            ps = fill.tile([P, FPC], F32, tag="fill")
            for dk in range(DC):
                nc.tensor.matmul(
                    ps[:],
                    lhsT=xsb[qb % 2][:, dk, i * P:(i + 1) * P],
                    rhs=wv_sb[:, dk, :],
                    start=(dk == 0),
                    stop=(dk == DC - 1),
                )
            nc.vector.tensor_copy(
                vtm[:, nt, :, 0:HD],
                ps[:].rearrange("p (h d) -> p h d", h=HPC),
            )

        # Each group: ONE k-tile, two row-tiled (concurrent) matmuls into a
        # double-buffered 2-bank PSUM tile — bank-separated per head (two
        # concurrent row-tile MMs sharing a PSUM bank wedge the PE), and
        # bufs=2 decouples the exp of group g from the scores of g+1.
        def emit_group(qb, m2, kind, j_or_kt):
            sc = scps.tile([P, 2, QB], F32, tag="sc")
            es = esp.tile([P, 2, QB], BF16, tag="es")
            q = qcur[qb % 2]
            if kind == "full":
                kt = j_or_kt
                for hh in range(2):
                    nc.tensor.matmul(
                        sc[:, hh, :],
                        lhsT=kT[hh * HD:(hh + 1) * HD, m2,
                                kt * P:(kt + 1) * P],
                        rhs=q[hh * HD:(hh + 1) * HD, m2, :],
                        start=True,
                        stop=True,
                    )
                nc.scalar.activation(es[:], sc[:], EXPF, scale=SCALE)
            else:
                j = j_or_kt
                ln = QB - j * P
                for hh in range(2):
                    nc.tensor.matmul(
                        sc[:, hh, 0:ln],
                        lhsT=kT[hh * HD:(hh + 1) * HD, m2,
                                (4 * qb + j) * P:(4 * qb + j + 1) * P],
                        rhs=q[hh * HD:(hh + 1) * HD, m2, j * P:QB],
                        start=True,
                        stop=True,
                        skip_group_check=True,
                    )
                nc.scalar.activation(es[:, :, 0:ln], sc[:, :, 0:ln],
                                     EXPF, scale=SCALE)
                for hh in range(2):
                    nc.gpsimd.tensor_mul(
                        es[:, hh, 0:P], es[:, hh, 0:P], dmask[:]
                    )
            return es

        def make_ctx(qb, m2, kind, j_or_kt, es, pctx, first, last):
            def emit():
                if kind == "full":
                    kt = j_or_kt
                    for hh in range(2):
                        nc.tensor.matmul(
                            pctx[hh][0:HD + 1, :],
                            lhsT=vtm[:, kt, 2 * m2 + hh, 0:HD + 1],
                            rhs=es[:, hh, :],
                            start=first[hh],
                            stop=False,
                            skip_group_check=True,
                        )
                        first[hh] = False
                else:
                    j = j_or_kt
                    ln = QB - j * P
                    for hh in range(2):
                        nc.tensor.matmul(
                            pctx[hh][0:HD + 1, j * P:QB],
                            lhsT=vtm[:, 4 * qb + j, 2 * m2 + hh, 0:HD + 1],
                            rhs=es[:, hh, 0:ln],
                            start=first[hh],
                            stop=(j == 3),
                            skip_group_check=True,
                        )
                        first[hh] = False
                if not last:
                    return
                for hh in range(2):
                    eng = nc.vector if hh == 0 else nc.scalar
                    cp = (nc.vector.tensor_copy if hh == 0
                          else nc.scalar.copy)
                    cp(
                        ctxU[qb % 2][hh * HD:(hh + 1) * HD, m2, :],
                        pctx[hh][0:HD, :],
                    )
                    dtmp = dtmpp.tile([P, QB], F32, tag="dtmp")
                    cp(dtmp[HD:HD + 1, :], pctx[hh][HD:HD + 1, :])
                    if qb == NQB - 1:
                        idx = m2 * 32 + hh
                        nc.sync.dma_start(
                            denstP[idx:idx + 1, :], dtmp[HD:HD + 1, :]
                        )
                    else:
                        idx = qb * 32 + 2 * m2 + hh
                        nc.sync.dma_start(
                            denst[idx:idx + 1, :], dtmp[HD:HD + 1, :]
                        )
            return emit

        pending = [None]

        def fire():
            if pending[0] is not None:
                pending[0]()
                pending[0] = None

        # ---- main fused loop ---------------------------------------------
        # Software pipeline at qb granularity: qb's attention runs while
        # qb-1's epilogue chain (den -> recip -> broadcast -> normalize)
        # and out-projection interleave as PE filler, and qb+1's
        # q-projection is emitted before qb's epilogue so the PE queue
        # never stalls on the cross-engine chain.
        rrows = [None, None]

        def emit_epi_chain(qb, m2):
            # DVE/DMA-only part: reciprocal, bf16 row, ship to partition HD
            rr = slice(m2 * 32, m2 * 32 + 2)
            rrow = rrows[qb % 2]
            nc.vector.reciprocal(recstP[rr, :], denstP[rr, :])
            nc.vector.tensor_copy(hiloP[rr, :], recstP[rr, :])
            nc.sync.dma_start(rrow[HD:HD + 1, 2 * m2:2 * m2 + 2, :],
                              hiloP[rr, :])

        def emit_epi_bc(qb, m2):
            # PE broadcast (K=1 outer product) + normalize into ctxT
            rrow = rrows[qb % 2]
            bc = fill.tile([P, QB], F32, tag="fill")
            for half in range(2):
                nc.tensor.matmul(
                    bc[half * HD:(half + 1) * HD, :],
                    lhsT=onesbf[HD:HD + 1, :],
                    rhs=rrow[HD:HD + 1, 2 * m2 + half, :],
                    start=True,
                    stop=True,
                )
            nc.vector.tensor_mul(
                ctxT[qb % 2][:, m2, :], ctxU[qb % 2][:, m2, :], bc[:]
            )

        def emit_epilogue(qb):
            r = slice(qb * 32, qb * 32 + HPC)
            rrow = rrows[qb % 2]
            nc.vector.reciprocal(recst[r, :], denst[r, :])
            nc.vector.tensor_copy(hilo[r, 0, :], recst[r, :])
            nc.sync.dma_start(rrow[HD:HD + 1, :, :], hilo[r, 0:1, :])
            for m2 in range(NQB):
                bc = fill.tile([P, QB], F32, tag="fill")
                for half in range(2):
                    nc.tensor.matmul(
                        bc[half * HD:(half + 1) * HD, :],
                        lhsT=onesbf[HD:HD + 1, :],
                        rhs=rrow[HD:HD + 1, 2 * m2 + half, :],
                        start=True,
                        stop=True,
                    )
                nc.vector.tensor_mul(
                    ctxT[qb % 2][:, m2, :], ctxU[qb % 2][:, m2, :], bc[:]
                )

        def make_outproj(qb, i):
            def emit():
                nt = 4 * qb + i
                pso = [fill.tile([P, QB], F32, tag="fill",
                                 name=f"o{nt}_{ec}") for ec in range(2)]
                for m in range(FPC // P):
                    for ec in range(2):
                        nc.tensor.matmul(
                            pso[ec][:],
                            lhsT=ctxT[qb % 2][:, m, i * P:(i + 1) * P],
                            rhs=wo_sb[:, m, ec * QB:(ec + 1) * QB],
                            start=(m == 0),
                            stop=(m == FPC // P - 1),
                        )
                stage = stagep.tile([P, D], F32, tag="stage")
                nc.vector.tensor_copy(stage[:, 0:QB], pso[0][:])
                nc.scalar.copy(stage[:, QB:D], pso[1][:])
                nc.sync.dma_start(out[nt * P:(nt + 1) * P, :], stage[:])
            return emit

        # Warmup matmuls on the triangular-mask tile: keep the PE busy
        # through the DMA lead-in so the HAM clock-gate opens (2.4 GHz)
        # before the first projection matmuls issue.
        wps = scps.tile([P, 2, QB], F32, tag="sc", name="warmup")
        for _ in range(30):
            nc.tensor.matmul(
                wps[:, 0, 0:P],
                lhsT=dmask[0:HD, :],
                rhs=dmask[0:HD, :],
                start=True,
                stop=True,
                skip_group_check=True,
            )

        qcur[0] = dbl.tile([P, NQB, QB], BF16, tag="qcur", name="qcur0")
        for m in range(NQB):
            proj_q(0, m)
        xsb[1] = dbl.tile([P, DC, QB], BF16, tag="xsb", name="xsb1")
        nc.sync.dma_start(
            xsb[1][:],
            xT[:, QB:2 * QB].rearrange("(c p) s -> p c s", p=P),
        )

        ep_prev = None
        op_prev = []
        for qb in range(NQB):
            ctxU[qb % 2] = dbl.tile([P, NQB, QB], F32, tag="ctxU",
                                    name=f"ctxU{qb}")
            ctxT[qb % 2] = dbl.tile([P, NQB, QB], BF16, tag="ctxT",
                                    name=f"ctxT{qb}")
            rrows[qb % 2] = dbl.tile([P, HPC, QB], BF16, tag="rrow",
                                     name=f"rrow{qb}")

            # k/v filler units for this qb, interleaved into pair-0's full
            # groups (the diagonal is the only consumer of qb's own k/v).
            fillers = [lambda m=m: proj_k(qb, m) for m in range(NQB)]
            fillers += [lambda i=i: proj_v(qb, i) for i in range(4)]

            lastqb = (qb == NQB - 1)
            for m2 in range(NQB):
                if lastqb and m2 >= 1:
                    fire()
                    emit_epi_chain(qb, m2 - 1)
                    if m2 >= 2:
                        emit_epi_bc(qb, m2 - 2)
                pctx = {
                    hh: ctxps.tile([P, QB], F32, tag=f"pctx{hh}",
                                   name=f"pctx{hh}")
                    for hh in range(2)
                }
                first = {0: True, 1: True}
                groups = [("full", kt) for kt in range(4 * qb)]
                groups += [("diag", j) for j in range(4)]
                nfull = len(groups) - 4
                for gi, (kind, jk) in enumerate(groups):
                    if m2 == 0 and gi == nfull:
                        for f in fillers:
                            f()
                        fillers = []
                    es = emit_group(qb, m2, kind, jk)
                    fire()
                    pending[0] = make_ctx(qb, m2, kind, jk, es, pctx,
                                          first, gi == len(groups) - 1)
                    if m2 == 0 and fillers and gi < nfull:
                        take = max(1, 8 // max(nfull, 1))
                        for _ in range(min(take, len(fillers))):
                            fillers.pop(0)()
                if m2 == 0 and ep_prev is not None:
                    ep_prev()
                    ep_prev = None
                if m2 >= 1 and op_prev:
                    op_prev.pop(0)()

            fire()
            for u in op_prev:
                u()
            op_prev = []

            if qb + 1 < NQB:
                qcur[(qb + 1) % 2] = dbl.tile([P, NQB, QB], BF16,
                                              tag="qcur",
                                              name=f"qcur{qb + 1}")
                for m in range(NQB):
                    proj_q(qb + 1, m)
                if qb + 2 < NQB:
                    xsb[(qb + 2) % 2] = dbl.tile([P, DC, QB], BF16,
                                                 tag="xsb",
                                                 name=f"xsb{qb + 2}")
                    nc.sync.dma_start(
                        xsb[(qb + 2) % 2][:],
                        xT[:, (qb + 2) * QB:(qb + 3) * QB].rearrange(
                            "(c p) s -> p c s", p=P
                        ),
                    )
                ep_prev = (lambda qb=qb: emit_epilogue(qb))
                op_prev = [make_outproj(qb, i) for i in range(4)]
            else:
                emit_epi_chain(qb, NQB - 1)
                emit_epi_bc(qb, NQB - 2)
                emit_epi_bc(qb, NQB - 1)
                for i in range(4):
                    make_outproj(qb, i)()

    dbl.release()
    persist.release()


_program_cache = None
last_results = None


def _get_program():
    global _program_cache
    if _program_cache is None:
        _program_cache = _build_program()
    return _program_cache


def kernel(x, Wq, Wk, Wv, Wo, bo):
    global last_results
    x = np.asarray(x, dtype=np.float32)
    Wq = np.asarray(Wq, dtype=np.float32)
    Wk = np.asarray(Wk, dtype=np.float32)
    Wv = np.asarray(Wv, dtype=np.float32)
    Wo = np.asarray(Wo, dtype=np.float32)
    bo = np.asarray(bo, dtype=np.float32)

    bf = ml_dtypes.bfloat16
    in_maps = []
    for c in range(NCORES):
        b, g = c // GPC, c % GPC
        fs = slice(g * FPC, (g + 1) * FPC)
        in_maps.append(
            {
                "xT": np.ascontiguousarray(x[b].T).astype(bf),
                "wq": np.ascontiguousarray(Wq[fs, :].T).astype(bf),
                "wk": np.ascontiguousarray(Wk[fs, :].T).astype(bf),
                "wv": np.ascontiguousarray(Wv[fs, :].T).astype(bf),
                "wo": np.ascontiguousarray(Wo[:, fs].T).astype(bf),
            }
        )

    nc = _get_program()
    res = run_bass_kernel_spmd(nc, in_maps, core_ids=list(range(NCORES)))
    last_results = res

    outf = np.empty((B, S, D), dtype=np.float32)
    for b in range(B):
        outf[b] = res.results[GPC * b]["out"] + res.results[GPC * b + 1]["out"] + bo
    return outf
